# revision 50
# baseline (speedup 1.0000x reference)
"""Trainium2 Bass kernel for the hard-negative-mining set loss (v6).

Structure:
  * host: positives/member tables are index bookkeeping on `target`
    (numpy), shipped as gather tables; xmem = member rows (the class
    shard of x), dxm = member target-logits, combo = enc - 2^24*mask.
  * device mining (row-sharded): per-class hardest negative via packed
    f32 key K = 8192*q + enc - 2^24*same_mask; q quantized -log prob by
    fp32 magic-constant rounding (adding 1.5*2^36 rounds to x8192).
  * cross-core combine: 4KB/core SBUF-to-SBUF remote_dma_broadcast
    exchange (XOR slot trick -- max() is order-invariant), gated on the
    kernel-entry barrier; no collective_compute data path.
  * class-sharded CE with factored softmax: E_m = exp(mem_m+mem_0-14)
    precomputed during the barrier gap; after the 128 neg rows arrive,
    rsum_m = sum(E_m * exp(neg)) via stt+accum. Only ~17us of work
    trails the exchange.
"""

import numpy as np

import concourse.bass as bass
import concourse.bacc as bacc
import concourse.tile as tile
from concourse import mybir
from concourse import bass_isa
from concourse.bass_utils import run_bass_kernel_spmd
from concourse.tile import add_dep_helper

B, C = 8192, 1024
NCORES = 8
BL = B // NCORES      # 1024 local mining rows per core
NT = BL // 128        # 8 row tiles
CT = C // 128         # 8 class blocks
CCL = C // NCORES     # 128 classes owned per core
M = B // C            # 8 members per class

SHIFT_A = 10.0        # mining softmax shift
SHIFT_C = 14.0        # summed-logits softmax shift
QSCALE = 140.0        # log-prob quantization: 1/140 nat resolution
SCALE = QSCALE * 8192.0            # 1146880.0
M2 = 1.5 * (2.0 ** 36)             # magic: ulp(M2) = 8192
M2C = M2 + SCALE * SHIFT_A         # exact multiple of 8192
MASKC = -16777216.0                # -2^24 same-class exclusion
F32 = mybir.dt.float32
I32 = mybir.dt.int32
OP = mybir.AluOpType
AF = mybir.ActivationFunctionType
AX = mybir.AxisListType


def build_nc():
    nc = bacc.Bacc("TRN2", target_bir_lowering=False, debug=False,
                   num_devices=NCORES)

    x_d = nc.dram_tensor("x", [B, C], F32, kind="ExternalInput")
    xloc_d = nc.dram_tensor("xloc", [BL, C], F32, kind="ExternalInput")
    xmem_d = nc.dram_tensor("xmem", [M * CCL, C], F32, kind="ExternalInput")
    combo_d = nc.dram_tensor("combo", [BL, C], F32, kind="ExternalInput")
    eqmc_d = nc.dram_tensor("eqmc", [128, C], F32, kind="ExternalInput")
    ident_d = nc.dram_tensor("ident", [128, 128], F32, kind="ExternalInput")
    ksel_d = nc.dram_tensor("ksel", [128, CT], F32, kind="ExternalInput")
    dxm_d = nc.dram_tensor("dxm", [128, M], F32, kind="ExternalInput")
    w8_d = nc.dram_tensor("w8", [128, M], F32, kind="ExternalInput")
    w7_d = nc.dram_tensor("w7", [128, M - 1], F32, kind="ExternalInput")
    # runtime wait target for the rdma exchange (16 on HW; walrus birsim
    # runs single-core with zeroed inputs, so its wait degenerates to >=0)
    mcw_d = nc.dram_tensor("mcw", [1, 1], I32, kind="ExternalInput")
    out_d = nc.dram_tensor("partial", [1, 1], F32, kind="ExternalOutput")

    bar_in = nc.dram_tensor("bar_in", [1, C], F32)
    bar_out = nc.dram_tensor("bar_out", [1, C], F32)

    with tile.TileContext(nc) as tc:
        with (
            tc.tile_pool(name="persist", bufs=1) as pp,
            tc.tile_pool(name="rscr", bufs=3) as rp,
            tc.tile_pool(name="kscr", bufs=3) as kp,
            tc.tile_pool(name="dumpC", bufs=2) as dcp,
            tc.tile_pool(name="small", bufs=6) as smp,
            tc.tile_pool(name="psB", bufs=1, space="PSUM") as psb,
        ):
            # entry barrier: small AllToAll triggered at queue head; its
            # completion proves every core is past its preamble sem-clear,
            # making the remote SBUF writes/sem-increments safe. Collectives
            # must read Internal DRAM, so seed bar_in on device first.
            barz = smp.tile([128, CT], F32, tag="barz")
            nc.vector.memset(barz, 0.0)
            nc.gpsimd.dma_start(out=bar_in.ap(), in_=barz)
            nc.gpsimd.collective_compute(
                "AllToAll", OP.bypass,
                replica_groups=[list(range(NCORES))],
                ins=[bar_in.ap().opt()], outs=[bar_out.ap().opt()])

            # ---------- mining-critical input DMAs (gpsimd queue:
            # its startup fence clears ~8us before sync's) ----------
            xloc = []
            for t in range(NT):
                xt = pp.tile([128, C], F32, tag=f"xloc{t}")
                nc.gpsimd.dma_start(out=xt, in_=xloc_d.ap()[t * 128:(t + 1) * 128, :])
                xloc.append(xt)
            combo = []
            for t in range(NT):
                cb = pp.tile([128, C], F32, tag=f"combo{t}")
                nc.gpsimd.dma_start(out=cb, in_=combo_d.ap()[t * 128:(t + 1) * 128, :])
                combo.append(cb)
            ident = pp.tile([128, 128], F32, tag="ident")
            nc.gpsimd.dma_start(out=ident, in_=ident_d.ap())
            ksel = pp.tile([128, CT], F32, tag="ksel")
            nc.sync.dma_start(out=ksel, in_=ksel_d.ap())
            dxm = pp.tile([128, M], F32, tag="dxm")
            nc.sync.dma_start(out=dxm, in_=dxm_d.ap())
            w8 = pp.tile([128, M], F32, tag="w8")
            nc.sync.dma_start(out=w8, in_=w8_d.ap())
            w7 = pp.tile([128, M - 1], F32, tag="w7")
            nc.sync.dma_start(out=w7, in_=w7_d.ap())
            mcw = smp.tile([1, 1], I32, tag="mcw")
            nc.sync.dma_start(out=mcw, in_=mcw_d.ap())

            ones = pp.tile([128, 1], F32, tag="ones")
            nc.vector.memset(ones, 1.0)
            shA = pp.tile([128, 1], F32, tag="shA")
            nc.vector.memset(shA, -SHIFT_A)
            shC = pp.tile([128, 1], F32, tag="shC")
            nc.vector.memset(shC, -SHIFT_C)

            # ---------- mining: packed-key build ----------
            dumpA = pp.tile([128, C], F32, tag="dumpA")
            rscat = smp.tile([128, NT], F32, tag="rscat")
            for t in range(NT):
                nc.scalar.activation(out=dumpA, in_=xloc[t], func=AF.Exp,
                                     bias=shA, scale=1.0,
                                     accum_out=rscat[:, t:t + 1])
            lrcat = smp.tile([128, NT], F32, tag="lrcat")
            ln_ins = nc.scalar.activation(out=lrcat, in_=rscat, func=AF.Ln)
            # b_t = f32(SCALE*lr + M2C): multiple of 8192 (carries lnrsum)
            btcat = smp.tile([128, NT], F32, tag="btcat")
            nc.vector.tensor_scalar(out=btcat, in0=lrcat, scalar1=SCALE,
                                    scalar2=M2C, op0=OP.mult, op1=OP.add)
            bt = [btcat[:, t:t + 1] for t in range(NT)]
            # r_t = Relu(-SCALE*x + b_t) = b_t + 8192*q (fp32 rounds @8192)
            # K_t = (r_t - b_t) + combo_t
            Kacc = pp.tile([128, C], F32, tag="Kacc")
            last_k = None
            for t in range(NT):
                rt = rp.tile([128, C], F32, tag="relu")
                relu_ins = nc.scalar.activation(out=rt, in_=xloc[t],
                                                func=AF.Relu,
                                                bias=bt[t], scale=-SCALE)
                if t == 0:
                    # pin ACT queue order: the Ln must precede the relus
                    # (relu -> btcat(DVE) -> Ln would deadlock otherwise)
                    add_dep_helper(relu_ins.ins, ln_ins.ins, sync=False)
                if t == 0:
                    last_k = nc.vector.scalar_tensor_tensor(
                        out=Kacc, in0=rt, scalar=bt[t], op0=OP.subtract,
                        in1=combo[t], op1=OP.add)
                else:
                    kt = kp.tile([128, C], F32, tag="kt")
                    nc.vector.scalar_tensor_tensor(
                        out=kt, in0=rt, scalar=bt[t], op0=OP.subtract,
                        in1=combo[t], op1=OP.add)
                    last_k = nc.vector.scalar_tensor_tensor(
                        out=Kacc, in0=Kacc, scalar=0.0, op0=OP.add,
                        in1=kt, op1=OP.max)

            # deferred input DMAs on the gpsimd queue: FIFO descriptor
            # order keeps these behind the xloc/combo transfers
            xmem = []
            last_xmem_dma = None
            for m in range(M):
                xt = pp.tile([128, C], F32, tag=f"xmem{m}")
                last_xmem_dma = nc.gpsimd.dma_start(
                    out=xt, in_=xmem_d.ap()[m * 128:(m + 1) * 128, :])
                xmem.append(xt)
            eqmc = pp.tile([128, C], F32, tag="eqmc")
            nc.gpsimd.dma_start(out=eqmc, in_=eqmc_d.ap())

            # local per-class max over partitions, then diagonal-extract
            # Ksend[p, ct] = Kpar[p, ct*128+p]
            Kpar = pp.tile([128, C], F32, tag="Kpar")
            par_ins = nc.gpsimd.partition_all_reduce(
                out_ap=Kpar, in_ap=Kacc, channels=128,
                reduce_op=bass_isa.ReduceOp.max)
            # order pin: all gpsimd-queue DMA dispatches before par, so the
            # engine queue can't stall dispatches behind par's DVE wait
            add_dep_helper(par_ins.ins, last_xmem_dma.ins, sync=False)
            Ksend = pp.tile([128, CT], F32, tag="Ksend")
            diag_ins = None
            for ct in range(CT):
                scri = kp.tile([128, 128], F32, tag="scri")
                diag_ins = nc.vector.scalar_tensor_tensor(
                    out=scri, in0=Kpar[:, ct * 128:(ct + 1) * 128],
                    scalar=1.0, op0=OP.mult, in1=ident, op1=OP.mult,
                    accum_out=Ksend[:, ct:ct + 1])

            # ---------- cross-core exchange (remote_dma broadcast) ----
            # slot i of Kall gets core (me XOR i)'s Ksend; max() below is
            # order-invariant so the XOR permutation is harmless.
            Kall = pp.tile([128, NCORES * CT], F32, tag="Kall")
            rsem = nc.alloc_semaphore(name="rdma_recv")
            lsem = nc.alloc_semaphore(name="rdma_sent")
            barld = smp.tile([1, 16], F32, tag="barld")
            bar_dma = nc.gpsimd.dma_start(out=barld,
                                          in_=bar_out.ap()[0:1, 0:16])
            for i in range(NCORES):
                rdests = [None] * NCORES
                rdests[i] = (0, i)
                nc.gpsimd.remote_dma_broadcast(
                    out_ap=Kall[:, i * CT:(i + 1) * CT],
                    in_ap=Ksend[:, 0:CT],
                    remote_sem=rsem, local_sem=lsem, rdests=rdests)
            trig = nc.gpsimd.trigger_dma(count=None)
            add_dep_helper(trig.ins, bar_dma.ins, sync=True)

            # ---------- overlapped with barrier/exchange ----------
            # psum_m = member_m + member_0 (pos pairing), then
            # E_m = exp(psum_m - SHIFT_C) for the factored CE
            emt = []
            last_ps = None
            for m in range(1, M):
                ps_ins = last_ps = nc.vector.scalar_tensor_tensor(
                    out=xmem[m], in0=xmem[m], scalar=0.0, op0=OP.add,
                    in1=xmem[0], op1=OP.add)
                et = pp.tile([128, C], F32, tag=f"emt{m}")
                e_ins = nc.scalar.activation(out=et, in_=xmem[m], func=AF.Exp,
                                             bias=shC, scale=1.0)
                emt.append(et)
                if m == 1:
                    # order pins: mining before the overlap work on both
                    # DVE (psum after last K) and ACT (E after last relu)
                    add_dep_helper(ps_ins.ins, last_k.ins, sync=False)
                    add_dep_helper(e_ins.ins, relu_ins.ins, sync=False)
            # tsh = sum_m w8_m * dxm_m  (host-gathered target logits)
            scr8 = smp.tile([128, M], F32, tag="scr8")
            tsh = smp.tile([128, 1], F32, tag="tsh")
            tsh_ins = nc.vector.scalar_tensor_tensor(
                out=scr8, in0=dxm, scalar=1.0, in1=w8,
                op0=OP.mult, op1=OP.mult, accum_out=tsh)

            # ---------- combine + decode + neg gather ----------
            wreg = nc.vector.alloc_register("rdma_wait_tgt")
            rld = nc.vector.reg_load(wreg, mcw[0:1, 0:1])
            wait_ins = nc.vector.wait_ge(rsem, wreg)
            add_dep_helper(wait_ins.ins, rld.ins, sync=False)
            # pin the wait after all pre-exchange DVE work, or the
            # scheduler hoists it to the queue head and deadlocks
            add_dep_helper(wait_ins.ins, diag_ins.ins, sync=False)
            add_dep_helper(wait_ins.ins, last_ps.ins, sync=False)
            add_dep_helper(wait_ins.ins, tsh_ins.ins, sync=False)
            # gmax[p, ct] = max over the 8 slots
            gsrc = bass.AP(tensor=Kall[:, 0:NCORES * CT].tensor,
                           offset=Kall[:, 0:NCORES * CT].offset,
                           ap=[Kall[:, 0:NCORES * CT].ap[0],
                               [1, CT], [CT, NCORES]])
            gmax = smp.tile([128, CT], F32, tag="gmax")
            red = nc.vector.tensor_reduce(out=gmax, in_=gsrc, axis=AX.X,
                                          op=OP.max)
            add_dep_helper(red.ins, wait_ins.ins, sync=False)
            # decode enc = gmax mod 8192 in exact f32, select owned col
            e1 = smp.tile([128, CT], F32, tag="e1")
            nc.vector.tensor_scalar(out=e1, in0=gmax, scalar1=1.0 / 8192.0,
                                    scalar2=None, op0=OP.mult)
            e2i = smp.tile([128, CT], I32, tag="e2i")
            nc.vector.tensor_copy(out=e2i, in_=e1)
            e3 = smp.tile([128, CT], F32, tag="e3")
            nc.vector.tensor_copy(out=e3, in_=e2i)
            e5 = smp.tile([128, CT], F32, tag="e5")
            nc.vector.scalar_tensor_tensor(out=e5, in0=e1, scalar=1.0,
                                           in1=e3, op0=OP.mult,
                                           op1=OP.subtract)
            nc.vector.tensor_scalar(out=e5, in0=e5, scalar1=8192.0,
                                    scalar2=None, op0=OP.mult)
            corr = smp.tile([128, CT], F32, tag="corr")
            nc.vector.tensor_scalar(out=corr, in0=e5, scalar1=0.0,
                                    scalar2=8192.0, op0=OP.is_lt, op1=OP.mult)
            nc.vector.tensor_tensor(out=e5, in0=e5, in1=corr, op=OP.add)
            rowf = smp.tile([128, CT], F32, tag="rowf")
            nc.vector.tensor_scalar(out=rowf, in0=e5, scalar1=-1.0,
                                    scalar2=8191.0, op0=OP.mult, op1=OP.add)
            # select this core's class block: rowsel = sum(rowf * ksel)
            scrk = smp.tile([128, CT], F32, tag="scrk")
            rowsel = smp.tile([128, 1], F32, tag="rowsel")
            nc.vector.scalar_tensor_tensor(out=scrk, in0=rowf, scalar=1.0,
                                           in1=ksel, op0=OP.mult,
                                           op1=OP.mult, accum_out=rowsel)
            rowi = smp.tile([128, 1], I32, tag="rowi")
            nc.vector.tensor_copy(out=rowi, in_=rowsel)
            negrow = pp.tile([128, C], F32, tag="negrow")
            for s in range(2):
                lo, hi = s * (C // 2), (s + 1) * (C // 2)
                nc.gpsimd.indirect_dma_start(
                    out=negrow[:, lo:hi], out_offset=None,
                    in_=x_d.ap(), element_offset=lo,
                    in_offset=bass.IndirectOffsetOnAxis(ap=rowi[:, 0:1],
                                                        axis=0))

            # ---------- factored CE tail ----------
            nrow = pp.tile([128, C], F32, tag="nrow")
            nrow_ins = nc.scalar.activation(out=nrow, in_=negrow, func=AF.Exp,
                                            bias=0.0, scale=1.0)
            rscat3 = smp.tile([128, M - 1], F32, tag="rscat3")
            scrd = dcp.tile([128, C], F32, tag="dumpC")
            for m in range(1, M):
                nc.vector.scalar_tensor_tensor(
                    out=scrd, in0=emt[m - 1], scalar=1.0, op0=OP.mult,
                    in1=nrow, op1=OP.mult,
                    accum_out=rscat3[:, m - 1:m])
            # dneg = neg[p, class(p)] via eqmc diag extraction
            scrC = dcp.tile([128, C], F32, tag="dumpC")
            dneg = smp.tile([128, 1], F32, tag="dneg")
            nc.vector.scalar_tensor_tensor(out=scrC, in0=negrow, scalar=1.0,
                                           in1=eqmc, op0=OP.mult, op1=OP.mult,
                                           accum_out=dneg)
            lcat = smp.tile([128, M - 1], F32, tag="lcat")
            lcat_ins = nc.scalar.activation(out=lcat, in_=rscat3, func=AF.Ln)
            add_dep_helper(lcat_ins.ins, nrow_ins.ins, sync=False)
            scr7 = smp.tile([128, M - 1], F32, tag="scr7")
            wl = smp.tile([128, 1], F32, tag="wl")
            nc.vector.scalar_tensor_tensor(out=scr7, in0=lcat, scalar=1.0,
                                           in1=w7, op0=OP.mult, op1=OP.mult,
                                           accum_out=wl)
            # loss_p = wl + 8*SHIFT_C - tsh - 8*dneg
            a1 = smp.tile([128, 1], F32, tag="a1")
            nc.vector.tensor_scalar(out=a1, in0=dneg, scalar1=-8.0,
                                    scalar2=8.0 * SHIFT_C,
                                    op0=OP.mult, op1=OP.add)
            nc.vector.tensor_tensor(out=a1, in0=a1, in1=wl, op=OP.add)
            nc.vector.tensor_tensor(out=a1, in0=a1, in1=tsh, op=OP.subtract)

            pss = psb.tile([1, 1], F32, tag="psum_out")
            nc.tensor.matmul(pss, lhsT=a1, rhs=ones, start=True, stop=True)
            outt = smp.tile([1, 1], F32, tag="outt")
            nc.vector.tensor_copy(out=outt, in_=pss)
            nc.sync.dma_start(out=out_d.ap(), in_=outt)

    nc.compile()
    return nc


_NC_CACHE = {}


def get_nc():
    if "nc" not in _NC_CACHE:
        _NC_CACHE["nc"] = build_nc()
    return _NC_CACHE["nc"]


def make_in_maps(x, target):
    x = np.ascontiguousarray(np.asarray(x, dtype=np.float32))
    tgt = np.asarray(target).astype(np.int64)
    assert x.shape == (B, C) and tgt.shape == (B,)

    eye = np.eye(C, dtype=np.float32)
    ident_full = np.eye(128, dtype=np.float32)

    # members[c] = sorted rows of class c (exactly M each)
    order = np.argsort(tgt, kind="stable")
    members = order.reshape(C, M).astype(np.int64)

    w8row = np.array([8.0, 2.0] + [1.0] * (M - 2), dtype=np.float32)
    w7row = np.array([2.0] + [1.0] * (M - 2), dtype=np.float32)
    w8_full = np.ascontiguousarray(np.broadcast_to(w8row, (128, M)))
    w7_full = np.ascontiguousarray(np.broadcast_to(w7row, (128, M - 1)))

    in_maps = []
    for k in range(NCORES):
        rows = slice(k * BL, (k + 1) * BL)
        tloc = tgt[rows]
        gi = k * BL + np.arange(BL)
        ck = np.arange(k * CCL, (k + 1) * CCL)
        mem_k = members[ck]                      # [128, M]
        xmem = np.ascontiguousarray(
            x[mem_k.T.reshape(-1)])              # [M*128, C], m-major
        dxm = np.ascontiguousarray(
            x[mem_k, ck[:, None]].astype(np.float32))   # [128, M]
        # combo[j, c] = (B-1 - global_row_j) - 2^24 * (target_j == c)
        combo = np.repeat((float(B) - 1.0 - gi).astype(np.float32)[:, None],
                          C, axis=1)
        combo[np.arange(BL), tloc] += MASKC
        kselb = np.zeros((128, CT), dtype=np.float32)
        kselb[:, k] = 1.0
        in_maps.append({
            "mcw": np.full((1, 1), 2 * NCORES, dtype=np.int32),
            "x": x,
            "xloc": np.ascontiguousarray(x[rows]),
            "xmem": xmem,
            "combo": np.ascontiguousarray(combo),
            "eqmc": np.ascontiguousarray(eye[ck]),
            "ident": ident_full,
            "ksel": kselb,
            "dxm": dxm,
            "w8": w8_full,
            "w7": w7_full,
        })
    return in_maps


def kernel(x, target):
    nc = get_nc()
    in_maps = make_in_maps(x, target)
    res = run_bass_kernel_spmd(nc, in_maps, core_ids=list(range(NCORES)))
    total = sum(float(res.results[k]["partial"][0, 0]) for k in range(NCORES))
    return np.float32(total / B)
